# revision 40
# baseline (speedup 1.0000x reference)
"""Trainium2 Bass kernel for gnn_message_passing (nn_FGL_2138893714004).

Reference computation:
    y = x * nf_weight                    # (8, 32, 50000)
    g = y[:, :, A]                       # (8, 32, 8192, 32)
    red = max(g, axis=-1)                # (8, 32, 8192)
    out = einsum('nio,ik->nko', red, ft) # (8, 64, 8192)
    out = out + bias                     # bias (64, 8192)

Strategy (8 NeuronCores): shard the 8192 output nodes 8 ways (1024 per core);
every core sees all 8 batch elements.  The host packs a token-major table
rows[j] = [x[0,:,j] .. x[7,:,j], nf[:,j], pad] (384 bf16 = 768 B) compacted
per core via np.unique (so indices fit dma_gather's int16).

On-core, the 1024 outputs are processed as 8 blocks of 128 (partition p =
output).  Each block's 32-neighbor set is fetched by `nsub` small SWDGE
dma_gathers (query q = d*128+p lands at [p, d, row]), sized to fit the
~64-desc/lane SWDGE ring so Q7 descriptor emission never throttles on its
own drain and the 4 queue core-pairs emit concurrently (emission at ~9 ns
per descriptor per pair is the pipeline pacer; single_packet=True helps).
Per sub-gather the DVE multiplies x-slices by the nf-slice (batch broadcast);
per block a 5-level out-of-place DVE max tree reduces d.  Tail per block:
PE transpose to value-major (full-128 identity — no tile_position quad mode,
which faults when interleaved with full loads; and no dma_start_transpose,
which Tile serializes against the SWDGE gathers, destroying overlap), then
block-diagonal matmuls (slab s covers batches 2s, 2s+1) into one PSUM tile,
DVE bias add, 2 HWDGE stores.
"""

import sys

sys.path.insert(0, "/opt/trn_rl_repo")

import ml_dtypes
import numpy as np

import concourse.bacc as bacc
import concourse.bass as _bass
import concourse.mybir as mybir
from concourse.bass_utils import run_bass_kernel_spmd
from concourse.tile import TileContext

N, INC, INN = 8, 32, 50000
OUTC, OUTN, D = 64, 8192, 32
NCORES = 8
O_SH = OUTN // NCORES          # 1024 output nodes per core
ROW = 384                      # table row: 256 x + 32 nf + 96 pad (bf16)
NDAT = N * INC + INC           # 288 real elements per row
VCAP = 32768                   # compacted table capacity (int16 index range)
OBLK = 128                     # outputs per block
NBLK = O_SH // OBLK            # 8 blocks
BF16 = mybir.dt.bfloat16
FP32 = mybir.dt.float32

_cache: dict = {}


def _relax_gather_elem_assert():
    """Allow non-256B-multiple elem_size for transpose=False dma_gather.

    The 256-byte restriction in bass.dma_gather is only needed for the
    transpose path (verified on hardware); relaxing it lets us gather the
    packed 576 B payload out of 768 B-strided table rows.  Falls back to
    padded 768 B rows if the bass source drifts.
    """
    if _cache.get("relaxed") is not None:
        return _cache["relaxed"]
    import inspect
    import textwrap
    try:
        fsrc = textwrap.dedent(inspect.getsource(_bass.BassGpSimd.dma_gather))
        old = ("        assert (\n"
               "            elem_size_bytes > 0 and elem_size_bytes % 256 == 0\n"
               "        )  # transpose restriction\n")
        new = ("        assert elem_size_bytes > 0 and (\n"
               "            elem_size_bytes % 256 == 0 or not transpose\n"
               "        )\n")
        assert old in fsrc
        ns = vars(inspect.getmodule(_bass.BassGpSimd)).copy()
        exec(compile(fsrc.replace(old, new), "<dma_gather_patched>", "exec"), ns)
        _bass.BassGpSimd.dma_gather = ns["dma_gather"]
        _cache["relaxed"] = True
    except Exception:
        _cache["relaxed"] = False
    return _cache["relaxed"]


def _subs(nsub: int):
    """Per-block sub-gather spans [(d0, nd), ...] covering d=0..32."""
    if nsub == 11:               # ring-fit: 2x(3*8+1)=50 <= 64 descs/lane
        return [(i * 3, 3) for i in range(10)] + [(30, 2)]
    d = D // nsub
    return [(i * d, d) for i in range(nsub)]


def _build(reps: int = 1, stages: str = 'full', nq: int = 4, gb: int = 24,
           scratch: int = 16384, nsub: int = 8, sp: int = 1, prb: int = 2,
           qmap: int = 0, fold2: int = 0):
    PRB = prb
    subs = _subs(nsub)
    NSUBS = len(subs)
    ngath = NBLK * NSUBS
    if qmap:
        w = [1.0 / (4.6 + 0.2 * q) for q in range(nq)]
        cred = [0.0] * nq
        qseq = []
        for _ in range(ngath):
            for q in range(nq):
                cred[q] += w[q]
            pick = max(range(nq), key=lambda q: cred[q])
            cred[pick] -= 1.0
            qseq.append(pick)
    else:
        qseq = [g % nq for g in range(ngath)]
    DMAX = max(nd for _, nd in subs)
    packed = _relax_gather_elem_assert()
    gw = NDAT if packed else ROW   # gathered row width in sbuf
    nc = bacc.Bacc("TRN2", target_bir_lowering=False, debug=False,
                   num_devices=NCORES, num_swdge_queues=nq,
                   dynamic_dma_scratch_size=scratch)
    tab = nc.dram_tensor("tab", [VCAP, ROW], BF16, kind="ExternalInput")
    idx = nc.dram_tensor("idx", [128, NBLK * NSUBS, DMAX * 8], mybir.dt.int16,
                         kind="ExternalInput")
    ftw = nc.dram_tensor("ftw", [128, 2, 128], BF16, kind="ExternalInput")
    identm = nc.dram_tensor("identm", [128, 128], BF16, kind="ExternalInput")
    bias_s = nc.dram_tensor("bias_s", [128, O_SH], FP32, kind="ExternalInput")
    out = nc.dram_tensor("out", [N, OUTC, O_SH], FP32, kind="ExternalOutput")

    with TileContext(nc, pool_alloc_mode="queue") as tc:
        with (
            tc.tile_pool(name="persist", bufs=1) as pp,
            tc.tile_pool(name="g", bufs=gb) as gp,
            tc.tile_pool(name="prod", bufs=PRB) as prp,
            tc.tile_pool(name="fold", bufs=PRB) as fp_,
            tc.tile_pool(name="red", bufs=3) as rp,
            tc.tile_pool(name="rt", bufs=3) as rtp,
            tc.tile_pool(name="outs", bufs=3) as op,
            tc.tile_pool(name="pst", bufs=2, space="PSUM") as pstp,
            tc.tile_pool(name="psm", bufs=4, space="PSUM") as psmp,
        ):
            idx_sb = pp.tile([128, NBLK * NSUBS, DMAX * 8], mybir.dt.int16)
            for b in range(NBLK):
                sl = slice(b * NSUBS, (b + 1) * NSUBS)
                nc.sync.dma_start(out=idx_sb[:, sl, :], in_=idx[:, sl, :])
            ftw_sb = pp.tile([128, 2, 128], BF16)
            nc.sync.dma_start(out=ftw_sb[:], in_=ftw[:, :, :])
            bias_sb = pp.tile([128, NBLK, OBLK], FP32)
            nc.sync.dma_start(
                out=bias_sb[:],
                in_=bias_s[:, :].rearrange("k (b o) -> k b o", b=NBLK))
            ident = pp.tile([128, 128], BF16)
            nc.sync.dma_start(out=ident[:], in_=identm[:, :])
            for _rep in range(reps):
              for b in range(NBLK):
                # ---- gather block b in d-half subs: q = d*128+p -> [p,d,row]
                prod = prp.tile([128, D, N, INC], BF16, tag="prod")
                red = rp.tile([128, N * INC], BF16, tag="red")
                ftile = (fp_.tile([128, 30, N * INC], BF16, tag="f",
                                  name="ftile")
                         if fold2 and b == NBLK - 1 else None)
                for s, (d0, nd) in enumerate(subs):
                    nidx = nd * OBLK
                    g = gp.tile([128, DMAX, gw], BF16, tag="g")
                    if stages != 'compute':
                        nc.gpsimd.dma_gather(
                            g[:, 0:nd, :], tab[:, 0:gw],
                            idx_sb[:, b * NSUBS + s, 0:nidx // 16],
                            nidx, nidx, gw,
                            elem_step=ROW if packed else None,
                            single_packet=bool(sp),
                            queue_num=qseq[b * NSUBS + s],
                        )
                    else:
                        nc.vector.memset(g[:, 0:1, 0:1], 0.0)
                    if stages == 'gather':
                        continue
                    # multiply: prod[p, d, n, c] = x * nf (broadcast n)
                    g4 = g[:, 0:nd, 0:NDAT].rearrange(
                        "p d (n c) -> p d n c", n=N + 1)
                    xs = g4[:, :, 0:N, :]
                    nfs = g4[:, :, N:N + 1, :].to_broadcast(
                        [128, nd, N, INC])
                    nc.vector.tensor_tensor(
                        out=prod[:, d0:d0 + nd, :, :],
                        in0=xs, in1=nfs, op=mybir.AluOpType.mult)
                    if fold2 and b == NBLK - 1 and nd == 4:
                        # incremental: fold this sub's 4 rows, max into red
                        p2 = prod[:].rearrange("p d n c -> p d (n c)")
                        f = ftile
                        nc.vector.tensor_tensor(
                            out=f[:, 3 * s:3 * s + 2, :],
                            in0=p2[:, d0:d0 + 2, :], in1=p2[:, d0 + 2:d0 + 4, :],
                            op=mybir.AluOpType.max)
                        if s == 0:
                            nc.vector.tensor_tensor(
                                out=red[:], in0=f[:, 0, :], in1=f[:, 1, :],
                                op=mybir.AluOpType.max)
                        else:
                            nc.vector.tensor_tensor(
                                out=f[:, 3 * s + 2, :], in0=f[:, 3 * s, :],
                                in1=f[:, 3 * s + 1, :], op=mybir.AluOpType.max)
                            nc.vector.tensor_tensor(
                                out=red[:], in0=red[:], in1=f[:, 3 * s + 2, :],
                                op=mybir.AluOpType.max)
                if stages == 'gather':
                    continue
                if not (fold2 and b == NBLK - 1):
                    # ---- 5-level max tree over d (out-of-place) ----
                    f = fp_.tile([128, 30, N * INC], BF16, tag="f")
                    p2 = prod[:].rearrange("p d n c -> p d (n c)")
                    nc.vector.tensor_tensor(out=f[:, 0:16, :],
                                            in0=p2[:, 0:16, :],
                                            in1=p2[:, 16:32, :],
                                            op=mybir.AluOpType.max)
                    nc.vector.tensor_tensor(out=f[:, 16:24, :],
                                            in0=f[:, 0:8, :],
                                            in1=f[:, 8:16, :],
                                            op=mybir.AluOpType.max)
                    nc.vector.tensor_tensor(out=f[:, 24:28, :],
                                            in0=f[:, 16:20, :],
                                            in1=f[:, 20:24, :],
                                            op=mybir.AluOpType.max)
                    nc.vector.tensor_tensor(out=f[:, 28:30, :],
                                            in0=f[:, 24:26, :],
                                            in1=f[:, 26:28, :],
                                            op=mybir.AluOpType.max)
                    nc.vector.tensor_tensor(out=red[:], in0=f[:, 28, :],
                                            in1=f[:, 29, :],
                                            op=mybir.AluOpType.max)
                if stages in ('nogather_notail', 'gather_mulfold'):
                    continue
                # ---- tail: PE-transpose red [o, v] -> [v, o], matmul ----
                rts = []
                for t in range(2):
                    pst = pstp.tile([128, OBLK], BF16, tag="pst")
                    nc.tensor.transpose(
                        out=pst[:],
                        in_=red[:, t * 128:(t + 1) * 128],
                        identity=ident[:],
                    )
                    rt = rtp.tile([128, OBLK], BF16, tag=f"rt{t}")
                    nc.vector.tensor_copy(out=rt[:], in_=pst[:])
                    rts.append(rt)
                if stages == 'no_mm':
                    continue
                # ---- block-diag matmul: slab s=2t+h covers batches 2s,2s+1
                # pso[c= j*64+k, s, o] = sum_i ft[i,k]*red[n=2s+j, i, o] ----
                pso = psmp.tile([128, 4, OBLK], FP32, tag="pso")
                for h in range(2):
                    for t in range(2):
                        nc.tensor.matmul(
                            out=pso[:, 2 * t + h, :],
                            lhsT=ftw_sb[:, h, :],
                            rhs=rts[t][:],
                            start=True, stop=True,
                        )
                if stages == 'no_bias':
                    continue
                osb = op.tile([128, 4, OBLK], FP32, tag="osb")
                nc.vector.tensor_tensor(
                    out=osb[:], in0=pso[:],
                    in1=bias_sb[:, b:b + 1, :].to_broadcast([128, 4, OBLK]),
                    op=mybir.AluOpType.add)
                if stages == 'no_store':
                    continue
                # row r < 64: batch n = 2s (j=0, k=r); r >= 64: n = 2s+1
                for j in range(2):
                    nc.sync.dma_start(
                        out=out[:, :, b * OBLK:(b + 1) * OBLK]
                            .rearrange("(s j) k o -> j k s o", j=2)[j],
                        in_=osb[j * 64:(j + 1) * 64, :, :])

    nc.compile()
    return nc


def _prep(x, nf_weight, ft_weight, bias, A, nsub=2):
    subs = _subs(nsub)
    NSUBS = len(subs)
    DMAX = max(nd for _, nd in subs)
    bf = ml_dtypes.bfloat16
    rows = np.zeros((INN, ROW), dtype=bf)
    # token-major: rows[j] = [x[0,:,j] ... x[7,:,j], nf[:,j], pad]
    rows[:, :N * INC] = np.ascontiguousarray(
        x.transpose(2, 0, 1)).reshape(INN, N * INC).astype(bf)
    rows[:, N * INC:NDAT] = nf_weight.T.astype(bf)
    # block-diag lhsT per half h: rows (2h+j)*32..+32, cols j*64..+64 = ft
    ftw = np.zeros((128, 2, 128), dtype=bf)
    for h in range(2):
        for j in range(2):
            r0 = (2 * h + j) * INC
            ftw[r0:r0 + INC, h, j * OUTC:(j + 1) * OUTC] = ft_weight.astype(bf)

    in_maps = []
    for s in range(NCORES):
        A_s = A[s * O_SH:(s + 1) * O_SH]                 # (1024, 32)
        uniq, inv = np.unique(A_s, return_inverse=True)
        assert len(uniq) <= VCAP, len(uniq)
        tab = np.zeros((VCAP, ROW), dtype=bf)
        tab[:len(uniq)] = rows[uniq]
        remap = inv.reshape(A_s.shape).astype(np.int16)  # [o_loc, d]
        idx16 = np.zeros((128, NBLK * NSUBS, DMAX * 8), dtype=np.int16)
        for b in range(NBLK):
            for sub, (d0, nd) in enumerate(subs):
                # query q = d*128 + p  ->  token remap[b*128+p, d0+d]
                flat = remap[b * OBLK:(b + 1) * OBLK,
                             d0:d0 + nd].T.reshape(-1)
                # wrapped: position j -> [j%16, j//16]
                idx16[:16, b * NSUBS + sub, 0:nd * 8] = \
                    flat.reshape(nd * 8, 16).T
        idx16[16:] = np.tile(idx16[:16], (7, 1, 1))
        in_maps.append({
            "tab": tab,
            "idx": idx16,
            "ftw": ftw,
            "identm": np.eye(128, dtype=bf),
            "bias_s": np.ascontiguousarray(np.tile(
                bias[:, s * O_SH:(s + 1) * O_SH], (2, 1))).astype(np.float32),
        })
    return in_maps


def run(x, nf_weight, ft_weight, bias, A, reps=1, stages='full', nq=4, gb=24,
        scratch=16384, nsub=8, sp=1, prb=2, qmap=0, fold2=0, **run_kwargs):
    """Build (cached), run on 8 cores, reassemble. Returns (out, results)."""
    key = ("nc", reps, stages, nq, gb, scratch, nsub, sp, prb, qmap, fold2)
    if key not in _cache:
        _cache[key] = _build(reps, stages, nq, gb, scratch, nsub, sp, prb,
                             qmap, fold2)
    nc = _cache[key]
    in_maps = _prep(np.asarray(x), np.asarray(nf_weight),
                    np.asarray(ft_weight), np.asarray(bias), np.asarray(A),
                    nsub)
    res = run_bass_kernel_spmd(nc, in_maps, core_ids=list(range(NCORES)),
                               **run_kwargs)
    out = np.empty((N, OUTC, OUTN), dtype=np.float32)
    for s in range(NCORES):
        out[:, :, s * O_SH:(s + 1) * O_SH] = res.results[s]["out"]
    return out, res


def kernel(x, nf_weight, ft_weight, bias, A):
    out, _ = run(x, nf_weight, ft_weight, bias, A)
    return out
